# revision 30
# baseline (speedup 1.0000x reference)
"""Trainium2 Bass kernel for nn_MoELayer_27754078667461 (top-2 MoE, E=8).

Strategy (expert-parallel, sparse):
  - Host: gating (xf @ Wg + bg), softmax, top-2 -> (expert, weight) per token.
  - Host: gather tokens per expert, pad to shared capacity C (SPMD).
  - Device (8 cores, 1 expert each): y = gelu(x @ W1 + b1) @ W2, scaled by the
    per-token combine weight.  bf16 matmuls with fp32 PSUM accumulation.
  - Host: scatter-add per-expert outputs + combine-weighted b2 term.

The reference computes all 8 experts densely over all 8192 tokens; only the
top-2 experts per token contribute, so this does ~4x less matmul work and
splits it 8 ways.
"""

import numpy as np
import ml_dtypes

P = 128
D_MODEL = 1024
D_FF = 4096
N_EXPERTS = 8
TOP_K = 2
BATCH, SEQ = 4, 2048
T = BATCH * SEQ
DP = D_MODEL // P   # 8 contraction passes for x @ W1
FB = D_FF // P      # 32 ff blocks

TRACE = False        # test.py sets this for profiling runs
TRACE_KW = {}
W1_PIECES = 32       # DMA granularity for the W1 preload
W2_LATE = False      # emit W2 loads after chunk-0's first MM1 block

_cache = {}


def _chunks_for(C):
    # full 512-token chunks, ragged remainder last: the first chunk's MM1
    # then consumes W1 slower than the DMA delivers it (no PE starvation)
    chunks = []
    r = C
    while r >= 512:
        chunks.append(512)
        r -= 512
    if r:
        chunks.append(r)
    return chunks


def _build(C, act="Gelu", repeats=1):
    """Build the SPMD Bass module for per-core token capacity C."""
    import concourse.bass as bass
    import concourse.mybir as mybir
    import concourse.tile as tile
    from concourse import bacc

    fp32 = mybir.dt.float32
    bf16 = mybir.dt.bfloat16

    chunks = _chunks_for(C)
    nc = bacc.Bacc("TRN2", target_bir_lowering=False, debug=False,
                   num_devices=N_EXPERTS)

    # DRAM I/O.  Layouts (host-prepared):
    #   xt   [128, 8*C]   bf16: xt[p, 8*t0 + dp*L + j] = x[t0+j, dp*128+p]
    #                      for each token chunk (t0, L)
    #   w1   [128, 32768] bf16: w1[p, ffb*1024 + dp*128 + c] = W1[dp*128+p, ffb*128+c]
    #   w2   [128, 32768] bf16: w2[p, ffb*1024 + c]          = W2[ffb*128+p, c]
    #   b1   [128, 32]    fp32: b1[p, ffb] = b1_orig[ffb*128+p]
    #   sc   [128, C/128] fp32: sc[p, g] = combine_weight[g*128+p]
    #   y    [C, 1024]    fp32 output (already scaled by combine weight)
    NG = sum((L + P - 1) // P for L in chunks)  # token sub-blocks of <=128
    xt_d = nc.dram_tensor("xt", [P, 8 * C], bf16, kind="ExternalInput").ap()
    w1_d = nc.dram_tensor("w1", [P, FB * DP * P], bf16, kind="ExternalInput").ap()
    w2_d = nc.dram_tensor("w2", [P, FB * P * 8], bf16, kind="ExternalInput").ap()
    b1_d = nc.dram_tensor("b1", [P, FB], fp32, kind="ExternalInput").ap()
    sc_d = nc.dram_tensor("sc", [P, NG], fp32, kind="ExternalInput").ap()
    y_d = nc.dram_tensor("y", [C, D_MODEL], fp32, kind="ExternalOutput").ap()
    warm_d = nc.dram_tensor("warm", [1, 4], fp32, kind="ExternalOutput").ap()

    GELU = getattr(mybir.ActivationFunctionType, act)

    with tile.TileContext(nc) as tc:
        with (
            tc.tile_pool(name="wpool", bufs=1) as wpool,
            tc.tile_pool(name="xpool", bufs=2) as xpool,
            tc.tile_pool(name="apool", bufs=1) as apool,
            tc.tile_pool(name="ypool", bufs=2) as ypool,
            tc.tile_pool(name="ht_ps", bufs=2, space=bass.MemorySpace.PSUM) as htp,
            tc.tile_pool(name="y_ps", bufs=2, space=bass.MemorySpace.PSUM) as ypp,
            tc.tile_pool(name="w_ps", bufs=1, space=bass.MemorySpace.PSUM) as wps,
        ):
            w1_sb = wpool.tile([P, FB * DP * P], bf16, tag="w1")
            w2_sb = wpool.tile([P, FB * P * 8], bf16, tag="w2")
            b1_sb = wpool.tile([P, FB], fp32, tag="b1")
            sc_sb = wpool.tile([P, NG], fp32, tag="sc")

            def load_xt(t0, L):
                xt_t = xpool.tile([P, 8 * L], bf16, tag="xt")
                # split the chunk load across DMA queues
                for q in range(4):
                    nc.sync.dma_start(xt_t[:, q * 2 * L:(q + 1) * 2 * L],
                                      xt_d[:, 8 * t0 + q * 2 * L:
                                           8 * t0 + (q + 1) * 2 * L])
                return xt_t

            # PE warm-up burst: runs while the first DMAs land, keeps the
            # HAM clock-gate from starting the real matmuls at 1.2 GHz.
            warm_in = xpool.tile([P, 640], bf16, tag="warm")
            warm_ps = wps.tile([P, 512], fp32, tag="warmps")
            nc.vector.memset(warm_in[:], 0.0)
            for i in range(24):
                nc.tensor.matmul(warm_ps[:], warm_in[:, :128], warm_in[:, 128:640],
                                 start=(i == 0), stop=(i == 23))
            warm_sb = ypool.tile([P, 4], fp32, tag="warmsb")
            nc.vector.tensor_copy(warm_sb[:1, :], warm_ps[:1, :4])
            nc.sync.dma_start(warm_d[:, :], warm_sb[:1, :])

            # DMA emission order = need order: x chunk 0, W1 (blocks MM1),
            # b1/sc, then W2 (not needed until first MM2, ~50us in).
            xt_first = load_xt(0, chunks[0])
            for piece in range(W1_PIECES):
                w = FB * DP * P // W1_PIECES
                s = slice(piece * w, (piece + 1) * w)
                nc.sync.dma_start(w1_sb[:, s], w1_d[:, s])
            nc.sync.dma_start(b1_sb[:], b1_d[:])
            nc.sync.dma_start(sc_sb[:], sc_d[:])

            def load_w2():
                for ffb in range(FB):
                    s = slice(ffb * 1024, (ffb + 1) * 1024)
                    nc.sync.dma_start(w2_sb[:, s], w2_d[:, s])
            if not W2_LATE:
                load_w2()

            iters = [(rep, ci, L) for rep in range(repeats)
                     for ci, L in enumerate(chunks)]
            for rep, ci, L in iters:
                if ci == 0:
                    t0 = 0   # token offset
                    g = 0    # token sub-block index (for sc columns)
                xt_t = xt_first if (ci == 0 and rep == 0) else load_xt(t0, L)
                at_t = apool.tile([P, FB * L], bf16, tag="at")

                # h^T[ff, t] = sum_dp W1[dp,ff]^T x^T[dp, t]; gelu -> a^T (bf16)
                for ffb in range(FB):
                    ht = htp.tile([P, L], fp32, tag="ht")
                    for dp in range(DP):
                        nc.tensor.matmul(
                            ht[:],
                            w1_sb[:, ffb * 1024 + dp * P: ffb * 1024 + (dp + 1) * P],
                            xt_t[:, dp * L:(dp + 1) * L],
                            start=(dp == 0), stop=(dp == DP - 1),
                        )
                    nc.scalar.activation(
                        at_t[:, ffb * L:(ffb + 1) * L], ht[:], GELU,
                        bias=b1_sb[:, ffb:ffb + 1], scale=1.0,
                    )
                    if W2_LATE and rep == 0 and ci == 0 and ffb == 0:
                        load_w2()

                # y[t, dm] = sum_ffb a^T[ffb, t]^T W2[ffb, dm], scaled, to DRAM
                for ts in range((L + P - 1) // P):
                    m = min(P, L - ts * P)
                    yp = ypp.tile([P, D_MODEL], fp32, tag="yp")
                    for half in range(2):
                        for ffb in range(FB):
                            nc.tensor.matmul(
                                yp[:m, half * 512:(half + 1) * 512],
                                at_t[:, ffb * L + ts * P: ffb * L + ts * P + m],
                                w2_sb[:, ffb * 1024 + half * 512:
                                      ffb * 1024 + (half + 1) * 512],
                                start=(ffb == 0), stop=(ffb == FB - 1),
                            )
                    y_sb = ypool.tile([P, D_MODEL], fp32, tag="ysb")
                    nc.vector.tensor_scalar_mul(
                        y_sb[:m, :], yp[:m, :], sc_sb[:m, g:g + 1])
                    nc.sync.dma_start(y_d[t0 + ts * P: t0 + ts * P + m, :],
                                      y_sb[:m, :])
                    g += 1
                t0 += L

    nc.compile()
    return nc


def _routing(xf, Wg, bg):
    """fp32 gating matching the reference: softmax probs, top-2."""
    gate_logits = (xf @ Wg + bg).astype(np.float32)
    m = gate_logits.max(axis=-1, keepdims=True)
    e = np.exp(gate_logits - m)
    probs = e / e.sum(axis=-1, keepdims=True)
    idx = np.argsort(-probs, axis=-1, kind="stable")[:, :TOP_K]
    w = np.take_along_axis(probs, idx, axis=-1)
    return gate_logits, idx.astype(np.int64), w.astype(np.float32)


def _prepare(x, Wg, bg, W1, b1, W2, b2):
    x = np.asarray(x, dtype=np.float32)
    Wg = np.asarray(Wg, dtype=np.float32)
    bg = np.asarray(bg, dtype=np.float32)
    W1 = np.asarray(W1, dtype=np.float32)
    b1 = np.asarray(b1, dtype=np.float32)
    W2 = np.asarray(W2, dtype=np.float32)
    b2 = np.asarray(b2, dtype=np.float32)

    xf = x.reshape(T, D_MODEL)
    gate_logits, idx, w = _routing(xf, Wg, bg)

    # token lists per expert
    toks = [np.where((idx == e).any(axis=1))[0] for e in range(N_EXPERTS)]
    # combine weight of token t for expert e (a token hits an expert at most once)
    wmat = np.zeros((T, N_EXPERTS), np.float32)
    np.put_along_axis(wmat, idx, w, axis=1)

    C = max(max(len(t) for t in toks), 128)
    chunks = _chunks_for(C)

    xfT_bf = np.ascontiguousarray(xf.T).astype(ml_dtypes.bfloat16)  # [1024, T]

    in_maps = []
    for e in range(N_EXPERTS):
        tl = toks[e]
        xg = xfT_bf[:, tl]                                   # [1024, cnt_e]
        xt = np.zeros((P, 8 * C), ml_dtypes.bfloat16)
        t0 = 0
        for L in chunks:
            blk = np.zeros((D_MODEL, L), ml_dtypes.bfloat16)
            n = max(0, min(L, xg.shape[1] - t0))
            if n:
                blk[:, :n] = xg[:, t0:t0 + n]
            xt[:, 8 * t0: 8 * (t0 + L)] = (
                blk.reshape(DP, P, L).transpose(1, 0, 2).reshape(P, 8 * L))
            t0 += L
        w1r = (W1[e].reshape(DP, P, FB, P).transpose(1, 2, 0, 3)
               .reshape(P, FB * DP * P).astype(ml_dtypes.bfloat16))
        w2r = (W2[e].reshape(FB, P, D_MODEL).transpose(1, 0, 2)
               .reshape(P, FB * D_MODEL).astype(ml_dtypes.bfloat16))
        b1r = np.ascontiguousarray(b1[e].reshape(FB, P).T).astype(np.float32)
        sc = np.zeros(C, np.float32)
        sc[:len(tl)] = wmat[tl, e]
        NG = sum((L + P - 1) // P for L in chunks)
        scr = np.zeros((P, NG), np.float32)
        g = 0
        t0 = 0
        for L in chunks:
            for ts in range((L + P - 1) // P):
                m = min(P, L - ts * P)
                scr[:m, g] = sc[t0 + ts * P: t0 + ts * P + m]
                g += 1
            t0 += L
        in_maps.append({"xt": np.ascontiguousarray(xt), "w1": w1r, "w2": w2r,
                        "b1": b1r, "sc": scr})

    return in_maps, toks, wmat, C, gate_logits


def kernel(x, Wg, bg, W1, b1, W2, b2):
    from concourse.bass_utils import run_bass_kernel_spmd

    b2 = np.asarray(b2, dtype=np.float32)
    in_maps, toks, wmat, C, gate_logits = _prepare(x, Wg, bg, W1, b1, W2, b2)

    if C not in _cache:
        _cache[C] = _build(C)
    nc = _cache[C]

    res = run_bass_kernel_spmd(nc, in_maps, core_ids=list(range(N_EXPERTS)),
                               trace=TRACE, **TRACE_KW)
    kernel.last_results = res

    out = np.zeros((T, D_MODEL), np.float32)
    for e in range(N_EXPERTS):
        tl = toks[e]
        out[tl] += res.results[e]["y"][:len(tl)]
    out += wmat @ b2  # b2 contribution, exact in fp32
    return out.reshape(BATCH, SEQ, D_MODEL), gate_logits


# revision 33
# speedup vs baseline: 1.0102x; 1.0102x over previous
"""Trainium2 Bass kernel for nn_MoELayer_27754078667461 (top-2 MoE, E=8).

Strategy (expert-parallel, sparse):
  - Host: gating (xf @ Wg + bg), softmax, top-2 -> (expert, weight) per token.
  - Host: gather tokens per expert, pad to shared capacity C (SPMD).
  - Device (8 cores, 1 expert each): y = gelu(x @ W1 + b1) @ W2, scaled by the
    per-token combine weight.  bf16 matmuls with fp32 PSUM accumulation.
  - Host: scatter-add per-expert outputs + combine-weighted b2 term.

The reference computes all 8 experts densely over all 8192 tokens; only the
top-2 experts per token contribute, so this does ~4x less matmul work and
splits it 8 ways.
"""

import numpy as np
import ml_dtypes

P = 128
D_MODEL = 1024
D_FF = 4096
N_EXPERTS = 8
TOP_K = 2
BATCH, SEQ = 4, 2048
T = BATCH * SEQ
DP = D_MODEL // P   # 8 contraction passes for x @ W1
FB = D_FF // P      # 32 ff blocks

TRACE = False        # test.py sets this for profiling runs
TRACE_KW = {}
W1_PIECES = 32       # DMA granularity for the W1 preload
W2_LATE = False      # emit W2 loads after chunk-0's first MM1 block
WARM_MMS = 128       # PE warm-up matmuls (fill the W1-load window)

_cache = {}


def _chunks_for(C):
    # full 512-token chunks, ragged remainder last: the first chunk's MM1
    # then consumes W1 slower than the DMA delivers it (no PE starvation)
    chunks = []
    r = C
    while r >= 512:
        chunks.append(512)
        r -= 512
    if r:
        chunks.append(r)
    return chunks


def _build(C, act="Gelu", repeats=1):
    """Build the SPMD Bass module for per-core token capacity C."""
    import concourse.bass as bass
    import concourse.mybir as mybir
    import concourse.tile as tile
    from concourse import bacc

    fp32 = mybir.dt.float32
    bf16 = mybir.dt.bfloat16

    chunks = _chunks_for(C)
    nc = bacc.Bacc("TRN2", target_bir_lowering=False, debug=False,
                   num_devices=N_EXPERTS)

    # DRAM I/O.  Layouts (host-prepared):
    #   xt   [128, 8*C]   bf16: xt[p, 8*t0 + dp*L + j] = x[t0+j, dp*128+p]
    #                      for each token chunk (t0, L)
    #   w1   [128, 32768] bf16: w1[p, ffb*1024 + dp*128 + c] = W1[dp*128+p, ffb*128+c]
    #   w2   [128, 32768] bf16: w2[p, ffb*1024 + c]          = W2[ffb*128+p, c]
    #   b1   [128, 32]    fp32: b1[p, ffb] = b1_orig[ffb*128+p]
    #   sc   [128, C/128] fp32: sc[p, g] = combine_weight[g*128+p]
    #   y    [C, 1024]    fp32 output (already scaled by combine weight)
    NG = sum((L + P - 1) // P for L in chunks)  # token sub-blocks of <=128
    xt_d = nc.dram_tensor("xt", [P, 8 * C], bf16, kind="ExternalInput").ap()
    w1_d = nc.dram_tensor("w1", [P, FB * DP * P], bf16, kind="ExternalInput").ap()
    w2_d = nc.dram_tensor("w2", [P, FB * P * 8], bf16, kind="ExternalInput").ap()
    b1_d = nc.dram_tensor("b1", [P, FB], fp32, kind="ExternalInput").ap()
    sc_d = nc.dram_tensor("sc", [P, NG], fp32, kind="ExternalInput").ap()
    y_d = nc.dram_tensor("y", [C, D_MODEL], fp32, kind="ExternalOutput").ap()
    warm_d = nc.dram_tensor("warm", [1, 4], fp32, kind="ExternalOutput").ap()

    GELU = getattr(mybir.ActivationFunctionType, act)

    with tile.TileContext(nc) as tc:
        with (
            tc.tile_pool(name="wpool", bufs=1) as wpool,
            tc.tile_pool(name="xpool", bufs=2) as xpool,
            tc.tile_pool(name="apool", bufs=1) as apool,
            tc.tile_pool(name="ypool", bufs=2) as ypool,
            tc.tile_pool(name="ht_ps", bufs=2, space=bass.MemorySpace.PSUM) as htp,
            tc.tile_pool(name="y_ps", bufs=2, space=bass.MemorySpace.PSUM) as ypp,
            tc.tile_pool(name="w_ps", bufs=1, space=bass.MemorySpace.PSUM) as wps,
        ):
            w1_sb = wpool.tile([P, FB * DP * P], bf16, tag="w1")
            w2_sb = wpool.tile([P, FB * P * 8], bf16, tag="w2")
            b1_sb = wpool.tile([P, FB], fp32, tag="b1")
            sc_sb = wpool.tile([P, NG], fp32, tag="sc")

            def load_xt(t0, L):
                xt_t = xpool.tile([P, 8 * L], bf16, tag="xt")
                # split the chunk load across DMA queues
                for q in range(4):
                    nc.sync.dma_start(xt_t[:, q * 2 * L:(q + 1) * 2 * L],
                                      xt_d[:, 8 * t0 + q * 2 * L:
                                           8 * t0 + (q + 1) * 2 * L])
                return xt_t

            # PE warm-up burst: runs while the first DMAs land, keeps the
            # HAM clock-gate from starting the real matmuls at 1.2 GHz.
            warm_in = xpool.tile([P, 640], bf16, tag="warm")
            warm_ps = wps.tile([P, 512], fp32, tag="warmps")
            nc.vector.memset(warm_in[:], 0.0)
            for i in range(WARM_MMS):
                nc.tensor.matmul(warm_ps[:], warm_in[:, :128], warm_in[:, 128:640],
                                 start=(i == 0), stop=(i == WARM_MMS - 1))
            warm_sb = ypool.tile([P, 4], fp32, tag="warmsb")
            nc.vector.tensor_copy(warm_sb[:1, :], warm_ps[:1, :4])
            nc.sync.dma_start(warm_d[:, :], warm_sb[:1, :])

            # DMA emission order = need order: x chunk 0, W1 (blocks MM1),
            # b1/sc, then W2 (not needed until first MM2, ~50us in).
            xt_first = load_xt(0, chunks[0])
            for piece in range(W1_PIECES):
                w = FB * DP * P // W1_PIECES
                s = slice(piece * w, (piece + 1) * w)
                nc.sync.dma_start(w1_sb[:, s], w1_d[:, s])
            nc.sync.dma_start(b1_sb[:], b1_d[:])
            nc.sync.dma_start(sc_sb[:], sc_d[:])

            def load_w2():
                for ffb in range(FB):
                    s = slice(ffb * 1024, (ffb + 1) * 1024)
                    nc.sync.dma_start(w2_sb[:, s], w2_d[:, s])
            if not W2_LATE:
                load_w2()

            iters = [(rep, ci, L) for rep in range(repeats)
                     for ci, L in enumerate(chunks)]
            for rep, ci, L in iters:
                if ci == 0:
                    t0 = 0   # token offset
                    g = 0    # token sub-block index (for sc columns)
                xt_t = xt_first if (ci == 0 and rep == 0) else load_xt(t0, L)
                at_t = apool.tile([P, FB * L], bf16, tag="at")

                # h^T[ff, t] = sum_dp W1[dp,ff]^T x^T[dp, t]; gelu -> a^T (bf16)
                for ffb in range(FB):
                    ht = htp.tile([P, L], fp32, tag="ht")
                    for dp in range(DP):
                        nc.tensor.matmul(
                            ht[:],
                            w1_sb[:, ffb * 1024 + dp * P: ffb * 1024 + (dp + 1) * P],
                            xt_t[:, dp * L:(dp + 1) * L],
                            start=(dp == 0), stop=(dp == DP - 1),
                        )
                    nc.scalar.activation(
                        at_t[:, ffb * L:(ffb + 1) * L], ht[:], GELU,
                        bias=b1_sb[:, ffb:ffb + 1], scale=1.0,
                    )
                    if W2_LATE and rep == 0 and ci == 0 and ffb == 0:
                        load_w2()

                # y[t, dm] = sum_ffb a^T[ffb, t]^T W2[ffb, dm], scaled, to DRAM
                for ts in range((L + P - 1) // P):
                    m = min(P, L - ts * P)
                    yp = ypp.tile([P, D_MODEL], fp32, tag="yp")
                    for half in range(2):
                        for ffb in range(FB):
                            nc.tensor.matmul(
                                yp[:m, half * 512:(half + 1) * 512],
                                at_t[:, ffb * L + ts * P: ffb * L + ts * P + m],
                                w2_sb[:, ffb * 1024 + half * 512:
                                      ffb * 1024 + (half + 1) * 512],
                                start=(ffb == 0), stop=(ffb == FB - 1),
                            )
                    y_sb = ypool.tile([P, D_MODEL], fp32, tag="ysb")
                    nc.vector.tensor_scalar_mul(
                        y_sb[:m, :], yp[:m, :], sc_sb[:m, g:g + 1])
                    nc.sync.dma_start(y_d[t0 + ts * P: t0 + ts * P + m, :],
                                      y_sb[:m, :])
                    g += 1
                t0 += L

    nc.compile()
    return nc


def _routing(xf, Wg, bg):
    """fp32 gating matching the reference: softmax probs, top-2."""
    gate_logits = (xf @ Wg + bg).astype(np.float32)
    m = gate_logits.max(axis=-1, keepdims=True)
    e = np.exp(gate_logits - m)
    probs = e / e.sum(axis=-1, keepdims=True)
    idx = np.argsort(-probs, axis=-1, kind="stable")[:, :TOP_K]
    w = np.take_along_axis(probs, idx, axis=-1)
    return gate_logits, idx.astype(np.int64), w.astype(np.float32)


def _prepare(x, Wg, bg, W1, b1, W2, b2):
    x = np.asarray(x, dtype=np.float32)
    Wg = np.asarray(Wg, dtype=np.float32)
    bg = np.asarray(bg, dtype=np.float32)
    W1 = np.asarray(W1, dtype=np.float32)
    b1 = np.asarray(b1, dtype=np.float32)
    W2 = np.asarray(W2, dtype=np.float32)
    b2 = np.asarray(b2, dtype=np.float32)

    xf = x.reshape(T, D_MODEL)
    gate_logits, idx, w = _routing(xf, Wg, bg)

    # token lists per expert
    toks = [np.where((idx == e).any(axis=1))[0] for e in range(N_EXPERTS)]
    # combine weight of token t for expert e (a token hits an expert at most once)
    wmat = np.zeros((T, N_EXPERTS), np.float32)
    np.put_along_axis(wmat, idx, w, axis=1)

    C = max(max(len(t) for t in toks), 128)
    chunks = _chunks_for(C)

    xfT_bf = np.ascontiguousarray(xf.T).astype(ml_dtypes.bfloat16)  # [1024, T]

    in_maps = []
    for e in range(N_EXPERTS):
        tl = toks[e]
        xg = xfT_bf[:, tl]                                   # [1024, cnt_e]
        xt = np.zeros((P, 8 * C), ml_dtypes.bfloat16)
        t0 = 0
        for L in chunks:
            blk = np.zeros((D_MODEL, L), ml_dtypes.bfloat16)
            n = max(0, min(L, xg.shape[1] - t0))
            if n:
                blk[:, :n] = xg[:, t0:t0 + n]
            xt[:, 8 * t0: 8 * (t0 + L)] = (
                blk.reshape(DP, P, L).transpose(1, 0, 2).reshape(P, 8 * L))
            t0 += L
        w1r = (W1[e].reshape(DP, P, FB, P).transpose(1, 2, 0, 3)
               .reshape(P, FB * DP * P).astype(ml_dtypes.bfloat16))
        w2r = (W2[e].reshape(FB, P, D_MODEL).transpose(1, 0, 2)
               .reshape(P, FB * D_MODEL).astype(ml_dtypes.bfloat16))
        b1r = np.ascontiguousarray(b1[e].reshape(FB, P).T).astype(np.float32)
        sc = np.zeros(C, np.float32)
        sc[:len(tl)] = wmat[tl, e]
        NG = sum((L + P - 1) // P for L in chunks)
        scr = np.zeros((P, NG), np.float32)
        g = 0
        t0 = 0
        for L in chunks:
            for ts in range((L + P - 1) // P):
                m = min(P, L - ts * P)
                scr[:m, g] = sc[t0 + ts * P: t0 + ts * P + m]
                g += 1
            t0 += L
        in_maps.append({"xt": np.ascontiguousarray(xt), "w1": w1r, "w2": w2r,
                        "b1": b1r, "sc": scr})

    return in_maps, toks, wmat, C, gate_logits


def kernel(x, Wg, bg, W1, b1, W2, b2):
    from concourse.bass_utils import run_bass_kernel_spmd

    b2 = np.asarray(b2, dtype=np.float32)
    in_maps, toks, wmat, C, gate_logits = _prepare(x, Wg, bg, W1, b1, W2, b2)

    if C not in _cache:
        _cache[C] = _build(C)
    nc = _cache[C]

    res = run_bass_kernel_spmd(nc, in_maps, core_ids=list(range(N_EXPERTS)),
                               trace=TRACE, **TRACE_KW)
    kernel.last_results = res

    out = np.zeros((T, D_MODEL), np.float32)
    for e in range(N_EXPERTS):
        tl = toks[e]
        out[tl] += res.results[e]["y"][:len(tl)]
    out += wmat @ b2  # b2 contribution, exact in fp32
    return out.reshape(BATCH, SEQ, D_MODEL), gate_logits


# revision 35
# speedup vs baseline: 1.0105x; 1.0003x over previous
"""Trainium2 Bass kernel for nn_MoELayer_27754078667461 (top-2 MoE, E=8).

Strategy (expert-parallel, sparse):
  - Host: gating (xf @ Wg + bg), softmax, top-2 -> (expert, weight) per token.
  - Host: gather tokens per expert, pad to shared capacity C (SPMD).
  - Device (8 cores, 1 expert each): y = gelu(x @ W1 + b1) @ W2, scaled by the
    per-token combine weight.  bf16 matmuls with fp32 PSUM accumulation.
  - Host: scatter-add per-expert outputs + combine-weighted b2 term.

The reference computes all 8 experts densely over all 8192 tokens; only the
top-2 experts per token contribute, so this does ~4x less matmul work and
splits it 8 ways.
"""

import numpy as np
import ml_dtypes

P = 128
D_MODEL = 1024
D_FF = 4096
N_EXPERTS = 8
TOP_K = 2
BATCH, SEQ = 4, 2048
T = BATCH * SEQ
DP = D_MODEL // P   # 8 contraction passes for x @ W1
FB = D_FF // P      # 32 ff blocks

TRACE = False        # test.py sets this for profiling runs
TRACE_KW = {}
W1_PIECES = 32       # DMA granularity for the W1 preload
W2_LATE = False      # emit W2 loads after chunk-0's first MM1 block
WARM_MMS = 128       # PE warm-up matmuls (fill the W1-load window)

_cache = {}


def _chunks_for(C):
    # full 512-token chunks, ragged remainder last: the first chunk's MM1
    # then consumes W1 slower than the DMA delivers it (no PE starvation)
    chunks = []
    r = C
    while r >= 512:
        chunks.append(512)
        r -= 512
    if r:
        chunks.append(r)
    return chunks


def _build(C, act="Gelu", repeats=1):
    """Build the SPMD Bass module for per-core token capacity C."""
    import concourse.bass as bass
    import concourse.mybir as mybir
    import concourse.tile as tile
    from concourse import bacc

    fp32 = mybir.dt.float32
    bf16 = mybir.dt.bfloat16

    chunks = _chunks_for(C)
    nc = bacc.Bacc("TRN2", target_bir_lowering=False, debug=False,
                   num_devices=N_EXPERTS)

    # DRAM I/O.  Layouts (host-prepared):
    #   xt   [128, 8*C]   bf16: xt[p, 8*t0 + dp*L + j] = x[t0+j, dp*128+p]
    #                      for each token chunk (t0, L)
    #   w1   [128, 32768] bf16: w1[p, ffb*1024 + dp*128 + c] = W1[dp*128+p, ffb*128+c]
    #   w2   [128, 32768] bf16: w2[p, ffb*1024 + c]          = W2[ffb*128+p, c]
    #   b1   [128, 32]    fp32: b1[p, ffb] = b1_orig[ffb*128+p]
    #   sc   [128, C/128] fp32: sc[p, g] = combine_weight[g*128+p]
    #   y    [C, 1024]    fp32 output (already scaled by combine weight)
    NG = sum((L + P - 1) // P for L in chunks)  # token sub-blocks of <=128
    xt_d = nc.dram_tensor("xt", [P, 8 * C], bf16, kind="ExternalInput").ap()
    w1_d = nc.dram_tensor("w1", [P, FB * DP * P], bf16, kind="ExternalInput").ap()
    w2_d = nc.dram_tensor("w2", [P, FB * P * 8], bf16, kind="ExternalInput").ap()
    b1_d = nc.dram_tensor("b1", [P, FB], fp32, kind="ExternalInput").ap()
    sc_d = nc.dram_tensor("sc", [P, NG], fp32, kind="ExternalInput").ap()
    y_d = nc.dram_tensor("y", [C, D_MODEL], fp32, kind="ExternalOutput").ap()
    warm_d = nc.dram_tensor("warm", [1, 4], fp32, kind="ExternalOutput").ap()

    GELU = getattr(mybir.ActivationFunctionType, act)

    with tile.TileContext(nc) as tc:
        with (
            tc.tile_pool(name="wpool", bufs=1) as wpool,
            tc.tile_pool(name="xpool", bufs=2) as xpool,
            tc.tile_pool(name="apool", bufs=1) as apool,
            tc.tile_pool(name="ypool", bufs=2) as ypool,
            tc.tile_pool(name="ht_ps", bufs=2, space=bass.MemorySpace.PSUM) as htp,
            tc.tile_pool(name="y_ps", bufs=2, space=bass.MemorySpace.PSUM) as ypp,
            tc.tile_pool(name="w_ps", bufs=1, space=bass.MemorySpace.PSUM) as wps,
        ):
            w1_sb = wpool.tile([P, FB * DP * P], bf16, tag="w1")
            w2_sb = wpool.tile([P, FB * P * 8], bf16, tag="w2")
            b1_sb = wpool.tile([P, FB], fp32, tag="b1")
            sc_sb = wpool.tile([P, NG], fp32, tag="sc")

            def load_xt(t0, L):
                xt_t = xpool.tile([P, 8 * L], bf16, tag="xt")
                # split the chunk load across DMA queues
                for q in range(4):
                    nc.sync.dma_start(xt_t[:, q * 2 * L:(q + 1) * 2 * L],
                                      xt_d[:, 8 * t0 + q * 2 * L:
                                           8 * t0 + (q + 1) * 2 * L])
                return xt_t

            # PE warm-up burst: runs while the first DMAs land, keeps the
            # HAM clock-gate from starting the real matmuls at 1.2 GHz.
            warm_in = xpool.tile([P, 640], bf16, tag="warm")
            warm_ps = wps.tile([P, 512], fp32, tag="warmps")
            nc.vector.memset(warm_in[:], 0.0)
            for i in range(WARM_MMS):
                nc.tensor.matmul(warm_ps[:], warm_in[:, :128], warm_in[:, 128:640],
                                 start=(i == 0), stop=(i == WARM_MMS - 1))
            warm_sb = ypool.tile([P, 4], fp32, tag="warmsb")
            nc.vector.tensor_copy(warm_sb[:1, :], warm_ps[:1, :4])
            nc.sync.dma_start(warm_d[:, :], warm_sb[:1, :])

            # DMA emission order = need order: x chunk 0, W1 (blocks MM1),
            # b1/sc, then W2 (not needed until first MM2, ~50us in).
            xt_first = load_xt(0, chunks[0])
            for piece in range(W1_PIECES):
                w = FB * DP * P // W1_PIECES
                s = slice(piece * w, (piece + 1) * w)
                nc.sync.dma_start(w1_sb[:, s], w1_d[:, s])
            nc.sync.dma_start(b1_sb[:], b1_d[:])
            nc.sync.dma_start(sc_sb[:], sc_d[:])

            def load_w2():
                for ffb in range(FB):
                    s = slice(ffb * 1024, (ffb + 1) * 1024)
                    nc.sync.dma_start(w2_sb[:, s], w2_d[:, s])
            if not W2_LATE:
                load_w2()

            iters = [(rep, ci, L) for rep in range(repeats)
                     for ci, L in enumerate(chunks)]
            for rep, ci, L in iters:
                if ci == 0:
                    t0 = 0   # token offset
                    g = 0    # token sub-block index (for sc columns)
                xt_t = xt_first if (ci == 0 and rep == 0) else load_xt(t0, L)
                at_t = apool.tile([P, FB * L], bf16, tag="at")

                # h^T[ff, t] = sum_dp W1[dp,ff]^T x^T[dp, t]; gelu -> a^T (bf16)
                for ffb in range(FB):
                    ht = htp.tile([P, L], fp32, tag="ht")
                    for dp in range(DP):
                        nc.tensor.matmul(
                            ht[:],
                            w1_sb[:, ffb * 1024 + dp * P: ffb * 1024 + (dp + 1) * P],
                            xt_t[:, dp * L:(dp + 1) * L],
                            start=(dp == 0), stop=(dp == DP - 1),
                        )
                    nc.scalar.activation(
                        at_t[:, ffb * L:(ffb + 1) * L], ht[:], GELU,
                        bias=b1_sb[:, ffb:ffb + 1], scale=1.0,
                    )
                    if W2_LATE and rep == 0 and ci == 0 and ffb == 0:
                        load_w2()

                # y[t, dm] = sum_ffb a^T[ffb, t]^T W2[ffb, dm], scaled, to DRAM.
                # The two dm-halves interleave so each aT tile is one weight
                # load for both; scale+store go out per half so the epilogue
                # of half 0 hides under the matmuls of half 1.
                n_ts = (L + P - 1) // P
                for ts in range(n_ts):
                    m = min(P, L - ts * P)
                    yp = ypp.tile([P, D_MODEL], fp32, tag="yp")
                    y_sb = ypool.tile([P, D_MODEL], fp32, tag="ysb")
                    last = (rep, ci, ts) == (repeats - 1, len(chunks) - 1,
                                             n_ts - 1)
                    if last:
                        # sequential halves: half-0's scale+store hides under
                        # half-1's matmuls, shrinking the kernel tail
                        order = [(half, ffb) for half in range(2)
                                 for ffb in range(FB)]
                    else:
                        # interleaved: each aT tile is one weight load for
                        # both dm-halves
                        order = [(half, ffb) for ffb in range(FB)
                                 for half in range(2)]
                    for half, ffb in order:
                        nc.tensor.matmul(
                            yp[:m, half * 512:(half + 1) * 512],
                            at_t[:, ffb * L + ts * P: ffb * L + ts * P + m],
                            w2_sb[:, ffb * 1024 + half * 512:
                                  ffb * 1024 + (half + 1) * 512],
                            start=(ffb == 0), stop=(ffb == FB - 1),
                            skip_group_check=True,
                        )
                        if last and half == 0 and ffb == FB - 1:
                            hs = slice(0, 512)
                            nc.vector.tensor_scalar_mul(
                                y_sb[:m, hs], yp[:m, hs], sc_sb[:m, g:g + 1])
                            nc.sync.dma_start(
                                y_d[t0 + ts * P: t0 + ts * P + m, hs],
                                y_sb[:m, hs])
                    if last:
                        hs = slice(512, 1024)
                        nc.vector.tensor_scalar_mul(
                            y_sb[:m, hs], yp[:m, hs], sc_sb[:m, g:g + 1])
                        nc.sync.dma_start(
                            y_d[t0 + ts * P: t0 + ts * P + m, hs], y_sb[:m, hs])
                    else:
                        nc.vector.tensor_scalar_mul(
                            y_sb[:m, :], yp[:m, :], sc_sb[:m, g:g + 1])
                        nc.sync.dma_start(y_d[t0 + ts * P: t0 + ts * P + m, :],
                                          y_sb[:m, :])
                    g += 1
                t0 += L

    nc.compile()
    return nc


def _routing(xf, Wg, bg):
    """fp32 gating matching the reference: softmax probs, top-2."""
    gate_logits = (xf @ Wg + bg).astype(np.float32)
    m = gate_logits.max(axis=-1, keepdims=True)
    e = np.exp(gate_logits - m)
    probs = e / e.sum(axis=-1, keepdims=True)
    idx = np.argsort(-probs, axis=-1, kind="stable")[:, :TOP_K]
    w = np.take_along_axis(probs, idx, axis=-1)
    return gate_logits, idx.astype(np.int64), w.astype(np.float32)


def _prepare(x, Wg, bg, W1, b1, W2, b2):
    x = np.asarray(x, dtype=np.float32)
    Wg = np.asarray(Wg, dtype=np.float32)
    bg = np.asarray(bg, dtype=np.float32)
    W1 = np.asarray(W1, dtype=np.float32)
    b1 = np.asarray(b1, dtype=np.float32)
    W2 = np.asarray(W2, dtype=np.float32)
    b2 = np.asarray(b2, dtype=np.float32)

    xf = x.reshape(T, D_MODEL)
    gate_logits, idx, w = _routing(xf, Wg, bg)

    # token lists per expert
    toks = [np.where((idx == e).any(axis=1))[0] for e in range(N_EXPERTS)]
    # combine weight of token t for expert e (a token hits an expert at most once)
    wmat = np.zeros((T, N_EXPERTS), np.float32)
    np.put_along_axis(wmat, idx, w, axis=1)

    C = max(max(len(t) for t in toks), 128)
    chunks = _chunks_for(C)

    xfT_bf = np.ascontiguousarray(xf.T).astype(ml_dtypes.bfloat16)  # [1024, T]

    in_maps = []
    for e in range(N_EXPERTS):
        tl = toks[e]
        xg = xfT_bf[:, tl]                                   # [1024, cnt_e]
        xt = np.zeros((P, 8 * C), ml_dtypes.bfloat16)
        t0 = 0
        for L in chunks:
            blk = np.zeros((D_MODEL, L), ml_dtypes.bfloat16)
            n = max(0, min(L, xg.shape[1] - t0))
            if n:
                blk[:, :n] = xg[:, t0:t0 + n]
            xt[:, 8 * t0: 8 * (t0 + L)] = (
                blk.reshape(DP, P, L).transpose(1, 0, 2).reshape(P, 8 * L))
            t0 += L
        w1r = (W1[e].reshape(DP, P, FB, P).transpose(1, 2, 0, 3)
               .reshape(P, FB * DP * P).astype(ml_dtypes.bfloat16))
        w2r = (W2[e].reshape(FB, P, D_MODEL).transpose(1, 0, 2)
               .reshape(P, FB * D_MODEL).astype(ml_dtypes.bfloat16))
        b1r = np.ascontiguousarray(b1[e].reshape(FB, P).T).astype(np.float32)
        sc = np.zeros(C, np.float32)
        sc[:len(tl)] = wmat[tl, e]
        NG = sum((L + P - 1) // P for L in chunks)
        scr = np.zeros((P, NG), np.float32)
        g = 0
        t0 = 0
        for L in chunks:
            for ts in range((L + P - 1) // P):
                m = min(P, L - ts * P)
                scr[:m, g] = sc[t0 + ts * P: t0 + ts * P + m]
                g += 1
            t0 += L
        in_maps.append({"xt": np.ascontiguousarray(xt), "w1": w1r, "w2": w2r,
                        "b1": b1r, "sc": scr})

    return in_maps, toks, wmat, C, gate_logits


def kernel(x, Wg, bg, W1, b1, W2, b2):
    from concourse.bass_utils import run_bass_kernel_spmd

    b2 = np.asarray(b2, dtype=np.float32)
    in_maps, toks, wmat, C, gate_logits = _prepare(x, Wg, bg, W1, b1, W2, b2)

    if C not in _cache:
        _cache[C] = _build(C)
    nc = _cache[C]

    res = run_bass_kernel_spmd(nc, in_maps, core_ids=list(range(N_EXPERTS)),
                               trace=TRACE, **TRACE_KW)
    kernel.last_results = res

    out = np.zeros((T, D_MODEL), np.float32)
    for e in range(N_EXPERTS):
        tl = toks[e]
        out[tl] += res.results[e]["y"][:len(tl)]
    out += wmat @ b2  # b2 contribution, exact in fp32
    return out.reshape(BATCH, SEQ, D_MODEL), gate_logits


# revision 38
# speedup vs baseline: 1.0214x; 1.0108x over previous
"""Trainium2 Bass kernel for nn_MoELayer_27754078667461 (top-2 MoE, E=8).

Strategy (expert-parallel, sparse):
  - Host: gating (xf @ Wg + bg), softmax, top-2 -> (expert, weight) per token.
  - Host: gather tokens per expert, pad to shared capacity C (SPMD).
  - Device (8 cores, 1 expert each): y = gelu(x @ W1 + b1) @ W2, scaled by the
    per-token combine weight.  bf16 matmuls with fp32 PSUM accumulation.
  - Host: scatter-add per-expert outputs + combine-weighted b2 term.

The reference computes all 8 experts densely over all 8192 tokens; only the
top-2 experts per token contribute, so this does ~4x less matmul work and
splits it 8 ways.
"""

import numpy as np
import ml_dtypes

P = 128
D_MODEL = 1024
D_FF = 4096
N_EXPERTS = 8
TOP_K = 2
BATCH, SEQ = 4, 2048
T = BATCH * SEQ
DP = D_MODEL // P   # 8 contraction passes for x @ W1
FB = D_FF // P      # 32 ff blocks

TRACE = False        # test.py sets this for profiling runs
TRACE_KW = {}
W1_PIECES = 32       # DMA granularity for the W1 preload
W2_LATE = False      # emit W2 loads after chunk-0's first MM1 block
WARM_MMS = 96        # PE warm-up matmuls (fill the W1-load window)
DMA_MIX = True       # split W1 preload across HWDGE + SWDGE queues

_cache = {}


def _chunks_for(C):
    # full 512-token chunks, ragged remainder last: the first chunk's MM1
    # then consumes W1 slower than the DMA delivers it (no PE starvation)
    chunks = []
    r = C
    while r >= 512:
        chunks.append(512)
        r -= 512
    if r:
        chunks.append(r)
    return chunks


def _build(C, act="Gelu", repeats=1):
    """Build the SPMD Bass module for per-core token capacity C."""
    import concourse.bass as bass
    import concourse.mybir as mybir
    import concourse.tile as tile
    from concourse import bacc

    fp32 = mybir.dt.float32
    bf16 = mybir.dt.bfloat16

    chunks = _chunks_for(C)
    nc = bacc.Bacc("TRN2", target_bir_lowering=False, debug=False,
                   num_devices=N_EXPERTS)

    # DRAM I/O.  Layouts (host-prepared):
    #   xt   [128, 8*C]   bf16: xt[p, 8*t0 + dp*L + j] = x[t0+j, dp*128+p]
    #                      for each token chunk (t0, L)
    #   w1   [128, 32768] bf16: w1[p, ffb*1024 + dp*128 + c] = W1[dp*128+p, ffb*128+c]
    #   w2   [128, 32768] bf16: w2[p, ffb*1024 + c]          = W2[ffb*128+p, c]
    #   b1   [128, 32]    fp32: b1[p, ffb] = b1_orig[ffb*128+p]
    #   sc   [128, C/128] fp32: sc[p, g] = combine_weight[g*128+p]
    #   y    [C, 1024]    fp32 output (already scaled by combine weight)
    NG = sum((L + P - 1) // P for L in chunks)  # token sub-blocks of <=128
    xt_d = nc.dram_tensor("xt", [P, 8 * C], bf16, kind="ExternalInput").ap()
    w1_d = nc.dram_tensor("w1", [P, FB * DP * P], bf16, kind="ExternalInput").ap()
    w2_d = nc.dram_tensor("w2", [P, FB * P * 8], bf16, kind="ExternalInput").ap()
    b1_d = nc.dram_tensor("b1", [P, FB], fp32, kind="ExternalInput").ap()
    sc_d = nc.dram_tensor("sc", [P, NG], fp32, kind="ExternalInput").ap()
    y_d = nc.dram_tensor("y", [C, D_MODEL], fp32, kind="ExternalOutput").ap()
    warm_d = nc.dram_tensor("warm", [1, 4], fp32, kind="ExternalOutput").ap()

    GELU = getattr(mybir.ActivationFunctionType, act)

    with tile.TileContext(nc) as tc:
        with (
            tc.tile_pool(name="wpool", bufs=1) as wpool,
            tc.tile_pool(name="xpool", bufs=2) as xpool,
            tc.tile_pool(name="apool", bufs=1) as apool,
            tc.tile_pool(name="ypool", bufs=2) as ypool,
            tc.tile_pool(name="ht_ps", bufs=2, space=bass.MemorySpace.PSUM) as htp,
            tc.tile_pool(name="y_ps", bufs=2, space=bass.MemorySpace.PSUM) as ypp,
            tc.tile_pool(name="w_ps", bufs=1, space=bass.MemorySpace.PSUM) as wps,
        ):
            w1_sb = wpool.tile([P, FB * DP * P], bf16, tag="w1")
            w2_sb = wpool.tile([P, FB * P * 8], bf16, tag="w2")
            b1_sb = wpool.tile([P, FB], fp32, tag="b1")
            sc_sb = wpool.tile([P, NG], fp32, tag="sc")

            def load_xt(t0, L):
                xt_t = xpool.tile([P, 8 * L], bf16, tag="xt")
                # split the chunk load across DMA queues
                for q in range(4):
                    nc.sync.dma_start(xt_t[:, q * 2 * L:(q + 1) * 2 * L],
                                      xt_d[:, 8 * t0 + q * 2 * L:
                                           8 * t0 + (q + 1) * 2 * L])
                return xt_t

            # PE warm-up burst: runs while the first DMAs land, keeps the
            # HAM clock-gate from starting the real matmuls at 1.2 GHz.
            warm_in = xpool.tile([P, 640], bf16, tag="warm")
            warm_ps = wps.tile([P, 512], fp32, tag="warmps")
            nc.vector.memset(warm_in[:], 0.0)
            for i in range(WARM_MMS):
                nc.tensor.matmul(warm_ps[:], warm_in[:, :128], warm_in[:, 128:640],
                                 start=(i == 0), stop=(i == WARM_MMS - 1))
            warm_sb = ypool.tile([P, 4], fp32, tag="warmsb")
            nc.vector.tensor_copy(warm_sb[:1, :], warm_ps[:1, :4])
            nc.sync.dma_start(warm_d[:, :], warm_sb[:1, :])

            # DMA emission order = need order: x chunk 0, W1 (blocks MM1),
            # b1/sc, then W2 (not needed until first MM2, ~50us in).
            xt_first = load_xt(0, chunks[0])
            for piece in range(W1_PIECES):
                w = FB * DP * P // W1_PIECES
                s = slice(piece * w, (piece + 1) * w)
                # alternate HWDGE/SWDGE so the critical W1 preload uses
                # both DMA queue families
                eng = nc.gpsimd if (DMA_MIX and piece % 2) else nc.sync
                eng.dma_start(w1_sb[:, s], w1_d[:, s])
            nc.sync.dma_start(b1_sb[:], b1_d[:])
            nc.sync.dma_start(sc_sb[:], sc_d[:])

            def load_w2():
                for ffb in range(FB):
                    s = slice(ffb * 1024, (ffb + 1) * 1024)
                    nc.sync.dma_start(w2_sb[:, s], w2_d[:, s])
            if not W2_LATE:
                load_w2()

            iters = [(rep, ci, L) for rep in range(repeats)
                     for ci, L in enumerate(chunks)]
            for rep, ci, L in iters:
                if ci == 0:
                    t0 = 0   # token offset
                    g = 0    # token sub-block index (for sc columns)
                xt_t = xt_first if (ci == 0 and rep == 0) else load_xt(t0, L)
                at_t = apool.tile([P, FB * L], bf16, tag="at")

                # h^T[ff, t] = sum_dp W1[dp,ff]^T x^T[dp, t]; gelu -> a^T (bf16)
                for ffb in range(FB):
                    ht = htp.tile([P, L], fp32, tag="ht")
                    for dp in range(DP):
                        nc.tensor.matmul(
                            ht[:],
                            w1_sb[:, ffb * 1024 + dp * P: ffb * 1024 + (dp + 1) * P],
                            xt_t[:, dp * L:(dp + 1) * L],
                            start=(dp == 0), stop=(dp == DP - 1),
                        )
                    nc.scalar.activation(
                        at_t[:, ffb * L:(ffb + 1) * L], ht[:], GELU,
                        bias=b1_sb[:, ffb:ffb + 1], scale=1.0,
                    )
                    if W2_LATE and rep == 0 and ci == 0 and ffb == 0:
                        load_w2()

                # y[t, dm] = sum_ffb a^T[ffb, t]^T W2[ffb, dm], scaled, to DRAM.
                # The two dm-halves interleave so each aT tile is one weight
                # load for both; scale+store go out per half so the epilogue
                # of half 0 hides under the matmuls of half 1.
                n_ts = (L + P - 1) // P
                for ts in range(n_ts):
                    m = min(P, L - ts * P)
                    yp = ypp.tile([P, D_MODEL], fp32, tag="yp")
                    y_sb = ypool.tile([P, D_MODEL], fp32, tag="ysb")
                    last = (rep, ci, ts) == (repeats - 1, len(chunks) - 1,
                                             n_ts - 1)
                    if last:
                        # sequential halves: half-0's scale+store hides under
                        # half-1's matmuls, shrinking the kernel tail
                        order = [(half, ffb) for half in range(2)
                                 for ffb in range(FB)]
                    else:
                        # interleaved: each aT tile is one weight load for
                        # both dm-halves
                        order = [(half, ffb) for ffb in range(FB)
                                 for half in range(2)]
                    for half, ffb in order:
                        nc.tensor.matmul(
                            yp[:m, half * 512:(half + 1) * 512],
                            at_t[:, ffb * L + ts * P: ffb * L + ts * P + m],
                            w2_sb[:, ffb * 1024 + half * 512:
                                  ffb * 1024 + (half + 1) * 512],
                            start=(ffb == 0), stop=(ffb == FB - 1),
                            skip_group_check=True,
                        )
                        if last and half == 0 and ffb == FB - 1:
                            hs = slice(0, 512)
                            nc.vector.tensor_scalar_mul(
                                y_sb[:m, hs], yp[:m, hs], sc_sb[:m, g:g + 1])
                            nc.sync.dma_start(
                                y_d[t0 + ts * P: t0 + ts * P + m, hs],
                                y_sb[:m, hs])
                    if last:
                        hs = slice(512, 1024)
                        nc.vector.tensor_scalar_mul(
                            y_sb[:m, hs], yp[:m, hs], sc_sb[:m, g:g + 1])
                        nc.sync.dma_start(
                            y_d[t0 + ts * P: t0 + ts * P + m, hs], y_sb[:m, hs])
                    else:
                        nc.vector.tensor_scalar_mul(
                            y_sb[:m, :], yp[:m, :], sc_sb[:m, g:g + 1])
                        nc.sync.dma_start(y_d[t0 + ts * P: t0 + ts * P + m, :],
                                          y_sb[:m, :])
                    g += 1
                t0 += L

    nc.compile()
    return nc


def _routing(xf, Wg, bg):
    """fp32 gating matching the reference: softmax probs, top-2."""
    gate_logits = (xf @ Wg + bg).astype(np.float32)
    m = gate_logits.max(axis=-1, keepdims=True)
    e = np.exp(gate_logits - m)
    probs = e / e.sum(axis=-1, keepdims=True)
    idx = np.argsort(-probs, axis=-1, kind="stable")[:, :TOP_K]
    w = np.take_along_axis(probs, idx, axis=-1)
    return gate_logits, idx.astype(np.int64), w.astype(np.float32)


def _prepare(x, Wg, bg, W1, b1, W2, b2):
    x = np.asarray(x, dtype=np.float32)
    Wg = np.asarray(Wg, dtype=np.float32)
    bg = np.asarray(bg, dtype=np.float32)
    W1 = np.asarray(W1, dtype=np.float32)
    b1 = np.asarray(b1, dtype=np.float32)
    W2 = np.asarray(W2, dtype=np.float32)
    b2 = np.asarray(b2, dtype=np.float32)

    xf = x.reshape(T, D_MODEL)
    gate_logits, idx, w = _routing(xf, Wg, bg)

    # token lists per expert
    toks = [np.where((idx == e).any(axis=1))[0] for e in range(N_EXPERTS)]
    # combine weight of token t for expert e (a token hits an expert at most once)
    wmat = np.zeros((T, N_EXPERTS), np.float32)
    np.put_along_axis(wmat, idx, w, axis=1)

    C = max(max(len(t) for t in toks), 128)
    chunks = _chunks_for(C)

    xfT_bf = np.ascontiguousarray(xf.T).astype(ml_dtypes.bfloat16)  # [1024, T]

    in_maps = []
    for e in range(N_EXPERTS):
        tl = toks[e]
        xg = xfT_bf[:, tl]                                   # [1024, cnt_e]
        xt = np.zeros((P, 8 * C), ml_dtypes.bfloat16)
        t0 = 0
        for L in chunks:
            blk = np.zeros((D_MODEL, L), ml_dtypes.bfloat16)
            n = max(0, min(L, xg.shape[1] - t0))
            if n:
                blk[:, :n] = xg[:, t0:t0 + n]
            xt[:, 8 * t0: 8 * (t0 + L)] = (
                blk.reshape(DP, P, L).transpose(1, 0, 2).reshape(P, 8 * L))
            t0 += L
        w1r = (W1[e].reshape(DP, P, FB, P).transpose(1, 2, 0, 3)
               .reshape(P, FB * DP * P).astype(ml_dtypes.bfloat16))
        w2r = (W2[e].reshape(FB, P, D_MODEL).transpose(1, 0, 2)
               .reshape(P, FB * D_MODEL).astype(ml_dtypes.bfloat16))
        b1r = np.ascontiguousarray(b1[e].reshape(FB, P).T).astype(np.float32)
        sc = np.zeros(C, np.float32)
        sc[:len(tl)] = wmat[tl, e]
        NG = sum((L + P - 1) // P for L in chunks)
        scr = np.zeros((P, NG), np.float32)
        g = 0
        t0 = 0
        for L in chunks:
            for ts in range((L + P - 1) // P):
                m = min(P, L - ts * P)
                scr[:m, g] = sc[t0 + ts * P: t0 + ts * P + m]
                g += 1
            t0 += L
        in_maps.append({"xt": np.ascontiguousarray(xt), "w1": w1r, "w2": w2r,
                        "b1": b1r, "sc": scr})

    return in_maps, toks, wmat, C, gate_logits


def kernel(x, Wg, bg, W1, b1, W2, b2):
    from concourse.bass_utils import run_bass_kernel_spmd

    b2 = np.asarray(b2, dtype=np.float32)
    in_maps, toks, wmat, C, gate_logits = _prepare(x, Wg, bg, W1, b1, W2, b2)

    if C not in _cache:
        _cache[C] = _build(C)
    nc = _cache[C]

    res = run_bass_kernel_spmd(nc, in_maps, core_ids=list(range(N_EXPERTS)),
                               trace=TRACE, **TRACE_KW)
    kernel.last_results = res

    out = np.zeros((T, D_MODEL), np.float32)
    for e in range(N_EXPERTS):
        tl = toks[e]
        out[tl] += res.results[e]["y"][:len(tl)]
    out += wmat @ b2  # b2 contribution, exact in fp32
    return out.reshape(BATCH, SEQ, D_MODEL), gate_logits


# revision 40
# speedup vs baseline: 1.0360x; 1.0143x over previous
"""Trainium2 Bass kernel for nn_MoELayer_27754078667461 (top-2 MoE, E=8).

Strategy (expert-parallel, sparse):
  - Host: gating (xf @ Wg + bg), softmax, top-2 -> (expert, weight) per token.
  - Host: gather tokens per expert, pad to shared capacity C (SPMD).
  - Device (8 cores, 1 expert each): y = gelu(x @ W1 + b1) @ W2, scaled by the
    per-token combine weight.  bf16 matmuls with fp32 PSUM accumulation.
  - Host: scatter-add per-expert outputs + combine-weighted b2 term.

The reference computes all 8 experts densely over all 8192 tokens; only the
top-2 experts per token contribute, so this does ~4x less matmul work and
splits it 8 ways.
"""

import numpy as np
import ml_dtypes

P = 128
D_MODEL = 1024
D_FF = 4096
N_EXPERTS = 8
TOP_K = 2
BATCH, SEQ = 4, 2048
T = BATCH * SEQ
DP = D_MODEL // P   # 8 contraction passes for x @ W1
FB = D_FF // P      # 32 ff blocks

TRACE = False        # test.py sets this for profiling runs
TRACE_KW = {}
W1_PIECES = 32       # DMA granularity for the W1 preload
W2_LATE = False      # emit W2 loads after chunk-0's first MM1 block
WARM_MMS = 80        # PE warm-up matmuls (fill the W1-load window)
DMA_MIX = True       # split W1 preload across HWDGE + SWDGE queues
XT0_MIX = True       # split the x chunk-0 load across both queue families
HT_BUFS = 3          # PSUM banks for the hT double-buffer
XT_BUFS = 2          # xt chunk prefetch depth

_cache = {}


def _chunks_for(C):
    # full 512-token chunks, ragged remainder last: the first chunk's MM1
    # then consumes W1 slower than the DMA delivers it (no PE starvation)
    chunks = []
    r = C
    while r >= 512:
        chunks.append(512)
        r -= 512
    if r:
        chunks.append(r)
    return chunks


def _build(C, act="Gelu", repeats=1):
    """Build the SPMD Bass module for per-core token capacity C."""
    import concourse.bass as bass
    import concourse.mybir as mybir
    import concourse.tile as tile
    from concourse import bacc

    fp32 = mybir.dt.float32
    bf16 = mybir.dt.bfloat16

    chunks = _chunks_for(C)
    nc = bacc.Bacc("TRN2", target_bir_lowering=False, debug=False,
                   num_devices=N_EXPERTS)

    # DRAM I/O.  Layouts (host-prepared):
    #   xt   [128, 8*C]   bf16: xt[p, 8*t0 + dp*L + j] = x[t0+j, dp*128+p]
    #                      for each token chunk (t0, L)
    #   w1   [128, 32768] bf16: w1[p, ffb*1024 + dp*128 + c] = W1[dp*128+p, ffb*128+c]
    #   w2   [128, 32768] bf16: w2[p, ffb*1024 + c]          = W2[ffb*128+p, c]
    #   b1   [128, 32]    fp32: b1[p, ffb] = b1_orig[ffb*128+p]
    #   sc   [128, C/128] fp32: sc[p, g] = combine_weight[g*128+p]
    #   y    [C, 1024]    fp32 output (already scaled by combine weight)
    NG = sum((L + P - 1) // P for L in chunks)  # token sub-blocks of <=128
    xt_d = nc.dram_tensor("xt", [P, 8 * C], bf16, kind="ExternalInput").ap()
    w1_d = nc.dram_tensor("w1", [P, FB * DP * P], bf16, kind="ExternalInput").ap()
    w2_d = nc.dram_tensor("w2", [P, FB * P * 8], bf16, kind="ExternalInput").ap()
    b1_d = nc.dram_tensor("b1", [P, FB], fp32, kind="ExternalInput").ap()
    sc_d = nc.dram_tensor("sc", [P, NG], fp32, kind="ExternalInput").ap()
    y_d = nc.dram_tensor("y", [C, D_MODEL], fp32, kind="ExternalOutput").ap()
    warm_d = nc.dram_tensor("warm", [1, 4], fp32, kind="ExternalOutput").ap()

    GELU = getattr(mybir.ActivationFunctionType, act)

    with tile.TileContext(nc) as tc:
        with (
            tc.tile_pool(name="wpool", bufs=1) as wpool,
            tc.tile_pool(name="xpool", bufs=XT_BUFS) as xpool,
            tc.tile_pool(name="apool", bufs=1) as apool,
            tc.tile_pool(name="ypool", bufs=2) as ypool,
            tc.tile_pool(name="ht_ps", bufs=HT_BUFS, space=bass.MemorySpace.PSUM) as htp,
            tc.tile_pool(name="y_ps", bufs=2, space=bass.MemorySpace.PSUM) as ypp,
            tc.tile_pool(name="w_ps", bufs=1, space=bass.MemorySpace.PSUM) as wps,
        ):
            w1_sb = wpool.tile([P, FB * DP * P], bf16, tag="w1")
            w2_sb = wpool.tile([P, FB * P * 8], bf16, tag="w2")
            b1_sb = wpool.tile([P, FB], fp32, tag="b1")
            sc_sb = wpool.tile([P, NG], fp32, tag="sc")

            def load_xt(t0, L, mix=False):
                xt_t = xpool.tile([P, 8 * L], bf16, tag="xt")
                # split the chunk load across DMA queues
                for q in range(4):
                    eng = nc.gpsimd if (mix and q % 2) else nc.sync
                    eng.dma_start(xt_t[:, q * 2 * L:(q + 1) * 2 * L],
                                  xt_d[:, 8 * t0 + q * 2 * L:
                                       8 * t0 + (q + 1) * 2 * L])
                return xt_t

            # PE warm-up burst: runs while the first DMAs land, keeps the
            # HAM clock-gate from starting the real matmuls at 1.2 GHz.
            warm_in = xpool.tile([P, 640], bf16, tag="warm")
            warm_ps = wps.tile([P, 512], fp32, tag="warmps")
            nc.vector.memset(warm_in[:], 0.0)
            for i in range(WARM_MMS):
                nc.tensor.matmul(warm_ps[:], warm_in[:, :128], warm_in[:, 128:640],
                                 start=(i == 0), stop=(i == WARM_MMS - 1))
            warm_sb = ypool.tile([P, 4], fp32, tag="warmsb")
            nc.vector.tensor_copy(warm_sb[:1, :], warm_ps[:1, :4])
            nc.sync.dma_start(warm_d[:, :], warm_sb[:1, :])

            # DMA emission order = need order: x chunk 0, W1 (blocks MM1),
            # b1/sc, then W2 (not needed until first MM2, ~50us in).
            xt_first = load_xt(0, chunks[0], mix=XT0_MIX)
            for piece in range(W1_PIECES):
                w = FB * DP * P // W1_PIECES
                s = slice(piece * w, (piece + 1) * w)
                # alternate HWDGE/SWDGE so the critical W1 preload uses
                # both DMA queue families
                eng = nc.gpsimd if (DMA_MIX and piece % 2) else nc.sync
                eng.dma_start(w1_sb[:, s], w1_d[:, s])
            nc.sync.dma_start(b1_sb[:], b1_d[:])
            nc.sync.dma_start(sc_sb[:], sc_d[:])

            def load_w2():
                for ffb in range(FB):
                    s = slice(ffb * 1024, (ffb + 1) * 1024)
                    nc.sync.dma_start(w2_sb[:, s], w2_d[:, s])
            if not W2_LATE:
                load_w2()

            iters = [(rep, ci, L) for rep in range(repeats)
                     for ci, L in enumerate(chunks)]
            for rep, ci, L in iters:
                if ci == 0:
                    t0 = 0   # token offset
                    g = 0    # token sub-block index (for sc columns)
                xt_t = xt_first if (ci == 0 and rep == 0) else load_xt(t0, L)
                at_t = apool.tile([P, FB * L], bf16, tag="at")

                # h^T[ff, t] = sum_dp W1[dp,ff]^T x^T[dp, t]; gelu -> a^T (bf16)
                for ffb in range(FB):
                    ht = htp.tile([P, L], fp32, tag="ht")
                    for dp in range(DP):
                        nc.tensor.matmul(
                            ht[:],
                            w1_sb[:, ffb * 1024 + dp * P: ffb * 1024 + (dp + 1) * P],
                            xt_t[:, dp * L:(dp + 1) * L],
                            start=(dp == 0), stop=(dp == DP - 1),
                        )
                    nc.scalar.activation(
                        at_t[:, ffb * L:(ffb + 1) * L], ht[:], GELU,
                        bias=b1_sb[:, ffb:ffb + 1], scale=1.0,
                    )
                    if W2_LATE and rep == 0 and ci == 0 and ffb == 0:
                        load_w2()

                # y[t, dm] = sum_ffb a^T[ffb, t]^T W2[ffb, dm], scaled, to DRAM.
                # The two dm-halves interleave so each aT tile is one weight
                # load for both; scale+store go out per half so the epilogue
                # of half 0 hides under the matmuls of half 1.
                n_ts = (L + P - 1) // P
                for ts in range(n_ts):
                    m = min(P, L - ts * P)
                    yp = ypp.tile([P, D_MODEL], fp32, tag="yp")
                    y_sb = ypool.tile([P, D_MODEL], fp32, tag="ysb")
                    last = (rep, ci, ts) == (repeats - 1, len(chunks) - 1,
                                             n_ts - 1)
                    if last:
                        # sequential halves: half-0's scale+store hides under
                        # half-1's matmuls, shrinking the kernel tail
                        order = [(half, ffb) for half in range(2)
                                 for ffb in range(FB)]
                    else:
                        # interleaved: each aT tile is one weight load for
                        # both dm-halves
                        order = [(half, ffb) for ffb in range(FB)
                                 for half in range(2)]
                    for half, ffb in order:
                        nc.tensor.matmul(
                            yp[:m, half * 512:(half + 1) * 512],
                            at_t[:, ffb * L + ts * P: ffb * L + ts * P + m],
                            w2_sb[:, ffb * 1024 + half * 512:
                                  ffb * 1024 + (half + 1) * 512],
                            start=(ffb == 0), stop=(ffb == FB - 1),
                            skip_group_check=True,
                        )
                        if last and half == 0 and ffb == FB - 1:
                            hs = slice(0, 512)
                            nc.vector.tensor_scalar_mul(
                                y_sb[:m, hs], yp[:m, hs], sc_sb[:m, g:g + 1])
                            nc.sync.dma_start(
                                y_d[t0 + ts * P: t0 + ts * P + m, hs],
                                y_sb[:m, hs])
                    if last:
                        hs = slice(512, 1024)
                        nc.vector.tensor_scalar_mul(
                            y_sb[:m, hs], yp[:m, hs], sc_sb[:m, g:g + 1])
                        nc.sync.dma_start(
                            y_d[t0 + ts * P: t0 + ts * P + m, hs], y_sb[:m, hs])
                    else:
                        nc.vector.tensor_scalar_mul(
                            y_sb[:m, :], yp[:m, :], sc_sb[:m, g:g + 1])
                        nc.sync.dma_start(y_d[t0 + ts * P: t0 + ts * P + m, :],
                                          y_sb[:m, :])
                    g += 1
                t0 += L

    nc.compile()
    return nc


def _routing(xf, Wg, bg):
    """fp32 gating matching the reference: softmax probs, top-2."""
    gate_logits = (xf @ Wg + bg).astype(np.float32)
    m = gate_logits.max(axis=-1, keepdims=True)
    e = np.exp(gate_logits - m)
    probs = e / e.sum(axis=-1, keepdims=True)
    idx = np.argsort(-probs, axis=-1, kind="stable")[:, :TOP_K]
    w = np.take_along_axis(probs, idx, axis=-1)
    return gate_logits, idx.astype(np.int64), w.astype(np.float32)


def _prepare(x, Wg, bg, W1, b1, W2, b2):
    x = np.asarray(x, dtype=np.float32)
    Wg = np.asarray(Wg, dtype=np.float32)
    bg = np.asarray(bg, dtype=np.float32)
    W1 = np.asarray(W1, dtype=np.float32)
    b1 = np.asarray(b1, dtype=np.float32)
    W2 = np.asarray(W2, dtype=np.float32)
    b2 = np.asarray(b2, dtype=np.float32)

    xf = x.reshape(T, D_MODEL)
    gate_logits, idx, w = _routing(xf, Wg, bg)

    # token lists per expert
    toks = [np.where((idx == e).any(axis=1))[0] for e in range(N_EXPERTS)]
    # combine weight of token t for expert e (a token hits an expert at most once)
    wmat = np.zeros((T, N_EXPERTS), np.float32)
    np.put_along_axis(wmat, idx, w, axis=1)

    C = max(max(len(t) for t in toks), 128)
    chunks = _chunks_for(C)

    xfT_bf = np.ascontiguousarray(xf.T).astype(ml_dtypes.bfloat16)  # [1024, T]

    in_maps = []
    for e in range(N_EXPERTS):
        tl = toks[e]
        xg = xfT_bf[:, tl]                                   # [1024, cnt_e]
        xt = np.zeros((P, 8 * C), ml_dtypes.bfloat16)
        t0 = 0
        for L in chunks:
            blk = np.zeros((D_MODEL, L), ml_dtypes.bfloat16)
            n = max(0, min(L, xg.shape[1] - t0))
            if n:
                blk[:, :n] = xg[:, t0:t0 + n]
            xt[:, 8 * t0: 8 * (t0 + L)] = (
                blk.reshape(DP, P, L).transpose(1, 0, 2).reshape(P, 8 * L))
            t0 += L
        w1r = (W1[e].reshape(DP, P, FB, P).transpose(1, 2, 0, 3)
               .reshape(P, FB * DP * P).astype(ml_dtypes.bfloat16))
        w2r = (W2[e].reshape(FB, P, D_MODEL).transpose(1, 0, 2)
               .reshape(P, FB * D_MODEL).astype(ml_dtypes.bfloat16))
        b1r = np.ascontiguousarray(b1[e].reshape(FB, P).T).astype(np.float32)
        sc = np.zeros(C, np.float32)
        sc[:len(tl)] = wmat[tl, e]
        NG = sum((L + P - 1) // P for L in chunks)
        scr = np.zeros((P, NG), np.float32)
        g = 0
        t0 = 0
        for L in chunks:
            for ts in range((L + P - 1) // P):
                m = min(P, L - ts * P)
                scr[:m, g] = sc[t0 + ts * P: t0 + ts * P + m]
                g += 1
            t0 += L
        in_maps.append({"xt": np.ascontiguousarray(xt), "w1": w1r, "w2": w2r,
                        "b1": b1r, "sc": scr})

    return in_maps, toks, wmat, C, gate_logits


def kernel(x, Wg, bg, W1, b1, W2, b2):
    from concourse.bass_utils import run_bass_kernel_spmd

    b2 = np.asarray(b2, dtype=np.float32)
    in_maps, toks, wmat, C, gate_logits = _prepare(x, Wg, bg, W1, b1, W2, b2)

    if C not in _cache:
        _cache[C] = _build(C)
    nc = _cache[C]

    res = run_bass_kernel_spmd(nc, in_maps, core_ids=list(range(N_EXPERTS)),
                               trace=TRACE, **TRACE_KW)
    kernel.last_results = res

    out = np.zeros((T, D_MODEL), np.float32)
    for e in range(N_EXPERTS):
        tl = toks[e]
        out[tl] += res.results[e]["y"][:len(tl)]
    out += wmat @ b2  # b2 contribution, exact in fp32
    return out.reshape(BATCH, SEQ, D_MODEL), gate_logits
